# revision 1
# baseline (speedup 1.0000x reference)
"""Multi-head causal attention with RoPE on 8 TRN2 NeuronCores.

Problem: B=2, S=2048, D=1024, H=16 heads, DH=64, fp32, causal, RoPE.

Sharding (hardcoded): core c in 0..7 handles batch b = c//4 and head group
g = c%4 (heads 4g..4g+3, channels 256g..256g+256). Each core computes its
4 heads end-to-end (QKV projections, RoPE, attention, its slice of the
output projection); the host sums the 4 partial output projections per
batch. RoPE tables replicated.

Device algorithm (per core), all matmuls in float32r (full-rate PE with
~1e-3-class rounding; fp32 PSUM accumulation):
  - load x^T [D,S]; project q^T,k^T per head pair [128,2048] (channels on
    partitions) and v in natural layout [s,c] with a riding ones column
    (v_ext) for softmax denominators.
  - RoPE applied in-place on q^T/k^T: half-rotation done with 4 SBUF->SBUF
    partition-shift DMAs per chunk, then 3 DVE ops (mul/mul/add) with
    host-precomputed cos / sign-folded-sin tables.
  - attention per head in transposed-score space: S^T[k,q] tiles from
    K=64 matmuls; exp via ACT (scale=1/8 fused, no max subtraction -- scores
    are O(5), exp is safe in fp32); causal handling: k-tiles above the
    diagonal are skipped, diagonal blocks are narrowed to their live columns
    and only the true-diagonal 128x128 slice gets a triangle mask multiply;
    AV via M=65 matmuls (ones column accumulates the denominator in PSUM
    row 64); normalize: denominator broadcast by a K=1 matmul, reciprocal
    on the broadcast (all lanes), multiply.
  - output projection y = attn @ Wo^T (this core's 256 channels only).
"""
import numpy as np

B, S, D, H = 2, 2048, 1024, 16
DH = 64
NCORES = 8
P = 128
QT = 512                  # q tile (free dim)
NQT = S // QT             # 4
NKT = S // P              # 16 k tiles
NE = D // P               # 8 contraction chunks
HPC = 4                   # heads per core
C = HPC * DH              # 256 channels per core

_cache = {}


def _attention(nc, qk_pair, v_ext, mask_sb, ones_sb, attnT,
               psS, psO, ptp, normp, MM, F32, MUL, EXP):
    vhs = [v_ext.rearrange("p t (h x) -> p t h x", h=HPC)[:, :, h]
           for h in range(HPC)]
    for qt in range(NQT):
        for hp in range(2):          # head pairs, 2-way interleaved chains
            hs = (2 * hp, 2 * hp + 1)
            qhs, khs, po = {}, {}, {}
            for h in hs:
                pr, half = h // 2, (h % 2) * DH
                qhs[h] = qk_pair[("q", pr)][half:half + DH]
                khs[h] = qk_pair[("k", pr)][half:half + DH]
                po[h] = psO.tile([DH + 1, QT], F32, tag="po", name=f"po{h}")
            nkt = 4 * qt + 4
            for kt in range(nkt):
                j = kt - 4 * qt   # >= 0 on diagonal blocks
                lo = max(j, 0) * P
                for h in hs:
                    ps = psS.tile([P, QT], F32, tag="ps", name="ps")[:, lo:]
                    nc.tensor.matmul(
                        ps,
                        lhsT=khs[h][:, kt * P:(kt + 1) * P],
                        rhs=qhs[h][:, qt * QT + lo:(qt + 1) * QT])
                    pt = ptp.tile([P, QT], MM, tag="pt", name="pt")[:, lo:]
                    nc.scalar.activation(pt, ps, EXP, scale=0.125)
                    if j >= 0:
                        nc.gpsimd.tensor_tensor(pt[:, :P], pt[:, :P],
                                                mask_sb, MUL)
                    nc.tensor.matmul(po[h][:, lo:], lhsT=vhs[h][:, kt],
                                     rhs=pt,
                                     start=(kt == 0), stop=(kt == nkt - 1))
            for h in hs:
                den = normp.tile([DH + 1, QT], MM, tag="den")
                nc.vector.tensor_copy(den[DH:DH + 1], po[h][DH:DH + 1])
                bc = psO.tile([DH + 1, QT], F32, tag="po", name="bc")[:DH]
                nc.tensor.matmul(bc, lhsT=ones_sb[DH:DH + 1],
                                 rhs=den[DH:DH + 1])
                bc_sb = normp.tile([DH, QT], F32, tag="bcs")
                with nc.allow_low_precision(reason="softmax denom recip"):
                    nc.vector.reciprocal(bc_sb, bc)
                nc.vector.tensor_tensor(
                    attnT[h][:, qt * QT:(qt + 1) * QT],
                    po[h][:DH], bc_sb, MUL)


def _build():
    import concourse.bass as bass
    import concourse.mybir as mybir
    import concourse.tile as tile
    from concourse import bacc

    MM = mybir.dt.float32r
    F32 = mybir.dt.float32
    MUL = mybir.AluOpType.mult
    ADD = mybir.AluOpType.add
    EXP = mybir.ActivationFunctionType.Exp

    nc = bacc.Bacc(trn_type="TRN2", target_bir_lowering=False, debug=False,
                   enable_asserts=False)
    xT = nc.dram_tensor("xT", [D, S], MM, kind="ExternalInput").ap()
    wq_t = nc.dram_tensor("wq_t", [D, C], MM, kind="ExternalInput").ap()
    wk_t = nc.dram_tensor("wk_t", [D, C], MM, kind="ExternalInput").ap()
    wv_t = nc.dram_tensor("wv_t", [D, C], MM, kind="ExternalInput").ap()
    wo4 = nc.dram_tensor("wo4", [DH, HPC, D], MM, kind="ExternalInput").ap()
    cos2 = nc.dram_tensor("cos2", [P, S], MM, kind="ExternalInput").ap()
    sin2 = nc.dram_tensor("sin2", [P, S], MM, kind="ExternalInput").ap()
    mask1 = nc.dram_tensor("mask1", [P, P], MM, kind="ExternalInput").ap()
    onesd = nc.dram_tensor("onesd", [P, DH], MM, kind="ExternalInput").ap()
    y = nc.dram_tensor("y", [S, D], F32, kind="ExternalOutput").ap()

    with tile.TileContext(nc) as tc:
        with tc.tile_pool(name="keep", bufs=1) as keep, \
             tc.tile_pool(name="ptp", bufs=6) as ptp, \
             tc.tile_pool(name="normp", bufs=2) as normp, \
             tc.tile_pool(name="work", bufs=3) as work, \
             tc.tile_pool(name="psS", bufs=2, space="PSUM") as psS, \
             tc.tile_pool(name="psO", bufs=4, space="PSUM") as psO:

            # ---------------- persistent tiles ----------------
            qk_pair = {(w, pr): keep.tile([P, S], MM, tag=f"{w}{pr}",
                                          name=f"{w}{pr}")
                       for w in ("q", "k") for pr in range(2)}
            v_ext = keep.tile([P, NKT, HPC * (DH + 1)], MM, tag="vext")
            mask_sb = keep.tile([P, P], MM, tag="mask")
            ones_sb = keep.tile([DH + 1, DH], MM, tag="ones")
            attnT = [keep.tile([DH, S], MM, tag=f"attnT{h}", name=f"attnT{h}")
                     for h in range(HPC)]
            wo_sb = keep.tile([DH, HPC, D], MM, tag="wo")

            # ---------------- phase 1: QKV + RoPE ----------------
            with tc.tile_pool(name="ph1", bufs=2) as ph1, \
                 tc.tile_pool(name="wts", bufs=1) as wts, \
                 tc.tile_pool(name="swapp", bufs=3) as swapp, \
                 tc.tile_pool(name="psQ", bufs=2, space="PSUM") as psQ:
                wq_sb = wts.tile([P, NE, C], MM, tag="wq")
                wk_sb = wts.tile([P, NE, C], MM, tag="wk")
                wv_sb = wts.tile([P, NE, C], MM, tag="wv")
                cos_sb = wts.tile([P, S], MM, tag="cos")
                sin_sb = wts.tile([P, S], MM, tag="sin")
                xts = []
                for e in range(NE):
                    xt0 = None if e else ph1.tile([P, NE, QT], MM, tag="xt",
                                                  name="xt0")
                    if e == 0:
                        xts.append(xt0)
                    nc.sync.dma_start(
                        xts[0][:, e],
                        xT[:, 0:QT].rearrange("(o p) s -> p o s", p=P)[:, e])
                    nc.sync.dma_start(
                        wq_sb[:, e],
                        wq_t.rearrange("(o p) c -> p o c", p=P)[:, e])
                    nc.sync.dma_start(
                        wk_sb[:, e],
                        wk_t.rearrange("(o p) c -> p o c", p=P)[:, e])
                    nc.sync.dma_start(
                        wv_sb[:, e],
                        wv_t.rearrange("(o p) c -> p o c", p=P)[:, e])
                nc.sync.dma_start(cos_sb, cos2)
                nc.sync.dma_start(sin_sb, sin2)
                w_of = {"q": wq_sb, "k": wk_sb}

                for st in range(NQT):  # s quarters of 512
                    if st == 0:
                        xt = xts[0]
                    else:
                        xt = ph1.tile([P, NE, QT], MM, tag="xt")
                        for e in range(NE):
                            nc.sync.dma_start(
                                xt[:, e], xT[:, st * QT:(st + 1) * QT]
                                .rearrange("(o p) s -> p o s", p=P)[:, e])
                    if st == 2:
                        # loads needed later (attention / output projection)
                        nc.sync.dma_start(mask_sb, mask1)
                        nc.sync.dma_start(ones_sb, onesd[:DH + 1])
                        nc.sync.dma_start(
                            v_ext.rearrange("p t (h x) -> p t h x",
                                            h=HPC)[:, :, :, DH:],
                            onesd.rearrange("p (t h) -> p t h",
                                            t=NKT)[:, :, :, None])
                        nc.sync.dma_start(wo_sb, wo4)
                    sl = slice(st * QT, (st + 1) * QT)
                    # q/k projections + rope, per head pair
                    for which in ("q", "k"):
                        for pr in range(2):
                            ps = psQ.tile([P, QT], F32, tag="ps")
                            for e in range(NE):
                                nc.tensor.matmul(
                                    ps,
                                    lhsT=w_of[which][:, e, pr * P:(pr + 1) * P],
                                    rhs=xt[:, e],
                                    start=(e == 0), stop=(e == NE - 1))
                            raw = qk_pair[(which, pr)][:, sl]
                            nc.vector.tensor_copy(raw, ps)
                            sw = swapp.tile([P, QT], MM, tag="swap")
                            for a in range(4):
                                src = (a ^ 1) * 32
                                nc.sync.dma_start(sw[a * 32:(a + 1) * 32],
                                                  raw[src:src + 32])
                            nc.vector.tensor_tensor(sw, sw, sin_sb[:, sl], MUL)
                            nc.vector.tensor_tensor(raw, raw, cos_sb[:, sl], MUL)
                            nc.vector.tensor_tensor(raw, raw, sw, ADD)
                    # v projection (natural layout, strided into v_ext)
                    for sb16 in range(4):
                        kt = st * 4 + sb16
                        pv = psQ.tile([P, QT], F32, tag="ps", name="pv")[:, :C]
                        for e in range(NE):
                            nc.tensor.matmul(
                                pv,
                                lhsT=xt[:, e, sb16 * P:(sb16 + 1) * P],
                                rhs=wv_sb[:, e],
                                start=(e == 0), stop=(e == NE - 1))
                        nc.vector.tensor_copy(
                            v_ext.rearrange("p t (h x) -> p t h x",
                                            h=HPC)[:, kt, :, :DH],
                            pv.rearrange("p (h x) -> p h x", h=HPC))

            # ---------------- phase 2: attention ----------------
            _attention(nc, qk_pair, v_ext, mask_sb, ones_sb, attnT,
                       psS, psO, ptp, normp, MM, F32, MUL, EXP)

            # ---------------- phase 3: output projection ----------------
            with tc.tile_pool(name="psY", bufs=2, space="PSUM") as psY:
                for sc in range(S // P):
                    for et in range(D // QT):
                        psy = psY.tile([P, QT], F32, tag="psy")
                        for h in range(HPC):
                            nc.tensor.matmul(
                                psy,
                                lhsT=attnT[h][:, sc * P:(sc + 1) * P],
                                rhs=wo_sb[:, h, et * QT:(et + 1) * QT],
                                start=(h == 0), stop=(h == HPC - 1))
                        y_sb = work.tile([P, QT], F32, tag="ysb")
                        if (sc + et) % 2 == 0:
                            nc.vector.tensor_copy(y_sb, psy)
                        else:
                            nc.scalar.copy(y_sb, psy)
                        nc.sync.dma_start(
                            y[sc * P:(sc + 1) * P, et * QT:(et + 1) * QT],
                            y_sb)
    nc.compile()
    return nc


def _get_nc():
    if "nc" not in _cache:
        _cache["nc"] = _build()
    return _cache["nc"]


def _host_inputs(x, Wq, Wk, Wv, Wo, cos, sin):
    """Build the 8 per-core input dicts."""
    cosT = np.ascontiguousarray(cos.T).astype(np.float32)     # [DH, S]
    sinT = np.ascontiguousarray(sin.T).astype(np.float32)
    sinS = np.concatenate([-sinT[:DH // 2], sinT[DH // 2:]], axis=0)
    cos2 = np.tile(cosT, (2, 1))                              # [128, S]
    sin2 = np.tile(sinS, (2, 1))
    mask1 = (np.arange(P)[:, None] <= np.arange(P)[None, :]).astype(np.float32)
    onesd = np.ones((P, DH), np.float32)

    in_maps = []
    for c in range(NCORES):
        b, g = divmod(c, 4)
        cs = slice(C * g, C * g + C)
        in_maps.append({
            "xT": np.ascontiguousarray(x[b].T).astype(np.float32),
            "wq_t": np.ascontiguousarray(Wq[cs].T).astype(np.float32),
            "wk_t": np.ascontiguousarray(Wk[cs].T).astype(np.float32),
            "wv_t": np.ascontiguousarray(Wv[cs].T).astype(np.float32),
            "wo4": np.ascontiguousarray(
                Wo.T[cs].reshape(HPC, DH, D).transpose(1, 0, 2)
            ).astype(np.float32),
            "cos2": cos2, "sin2": sin2, "mask1": mask1, "onesd": onesd,
        })
    return in_maps


def run(x, Wq, Wk, Wv, Wo, cos, sin, mask=None, trace=False, **trace_kw):
    import os
    import time
    if not trace:
        # The axon NTFF-profile hook is not installed in all containers;
        # make sure an inherited BASS_TRACE=1 can't send us down that path.
        os.environ.setdefault("BASS_NEVER_TRACE", "1")
    from concourse.bass_utils import run_bass_kernel_spmd
    nc = _get_nc()
    in_maps = _host_inputs(np.asarray(x), np.asarray(Wq), np.asarray(Wk),
                           np.asarray(Wv), np.asarray(Wo), np.asarray(cos),
                           np.asarray(sin))
    try:
        res = run_bass_kernel_spmd(nc, in_maps, core_ids=list(range(NCORES)),
                                   trace=trace, **trace_kw)
    except Exception:
        # one retry for transient device states (e.g. NRT_EXEC_UNIT errors)
        time.sleep(15)
        res = run_bass_kernel_spmd(nc, in_maps, core_ids=list(range(NCORES)),
                                   trace=trace, **trace_kw)
    parts = [r["y"] for r in res.results]
    out = np.stack([parts[0] + parts[1] + parts[2] + parts[3],
                    parts[4] + parts[5] + parts[6] + parts[7]])
    return out.astype(np.float32), res


def kernel(x, Wq, Wk, Wv, Wo, cos, sin, mask=None, **_):
    out, _res = run(x, Wq, Wk, Wv, Wo, cos, sin, mask)
    return out



# revision 9
# speedup vs baseline: 1.0840x; 1.0840x over previous
"""Multi-head causal attention with RoPE on 8 TRN2 NeuronCores.

Problem: B=2, S=2048, D=1024, H=16 heads, DH=64, fp32, causal, RoPE.

Sharding (hardcoded): core c in 0..7 handles batch b = c//4 and head group
g = c%4 (heads 4g..4g+3, channels 256g..256g+256). Each core computes its
4 heads end-to-end (QKV projections, RoPE, attention, its slice of the
output projection); the host sums the 4 partial output projections per
batch. RoPE tables replicated.

Device algorithm (per core), all matmuls in float32r (full-rate PE with
fp32 PSUM accumulation). Emission interleaves the three phases so PE work
of one stage overlaps DVE/ACT/DMA work of the neighbours; DMA instruction
count is kept low (each costs ~0.6us on the shared HWDGE generator and the
DMA engines are modeled as one serialized resource), and latency-critical
DMAs (RoPE shifts, next x chunk, Wo) issue from the ACT queue mid-phase so
they are serviced in arrival order ahead of bulk prefetches:
  - phase 1 (per 512-seq chunk st): q^T,k^T per head pair [128,2048]
    (channels on partitions); both pairs of one projection accumulate in
    a single [128,1024] PSUM slot, one ACT copy moves both to SBUF; RoPE
    half-rotation for q and k together via 4 partition-shift SBUF->SBUF
    DMAs on a [128,4,512] view, then 3 DVE ops per (proj, pair); v in
    natural layout [s,c] with a riding ones column for softmax
    denominators, one ACT copy per chunk.
  - attention (qt = st-1, overlapped): scores for the two heads of a pair
    land in one [128,1024] PSUM slot; exp via a single strided ACT op per
    k-tile (scale=1/8 fused, no max subtraction -- scores are O(5), exp is
    safe in fp32); causal handling: k-tiles above the diagonal skipped,
    diagonal blocks narrowed to live columns, true-diagonal 128x128 slices
    of both heads masked by one strided GPSIMD multiply; AV accumulates
    [65,512] fp32 PSUM per head (ones column gives the denominator in row
    64); normalize: reciprocal of the denominator row (DVE), GPSIMD
    partition-broadcast to 64 partitions, multiply (DVE) into per-pair
    attnT tiles [128,2048] (two heads stacked on partitions).
  - output projection (rows of qt, right after): per 128-row block, two
    K=128 matmuls (head pair stacked) per 512-wide slice accumulate
    y = attn @ Wo^T; one [128,1024] store per row block.
"""
import numpy as np

B, S, D, H = 2, 2048, 1024, 16
DH = 64
NCORES = 8
P = 128
QT = 512                  # q tile (free dim)
NQT = S // QT             # 4
NKT = S // P              # 16 k tiles
NE = D // P               # 8 contraction chunks
HPC = 4                   # heads per core
C = HPC * DH              # 256 channels per core

_cache = {}


def _build():
    import concourse.bass as bass
    import concourse.mybir as mybir
    import concourse.tile as tile
    from concourse import bacc

    MM = mybir.dt.float32r
    F32 = mybir.dt.float32
    MUL = mybir.AluOpType.mult
    ADD = mybir.AluOpType.add
    EXP = mybir.ActivationFunctionType.Exp

    nc = bacc.Bacc(trn_type="TRN2", target_bir_lowering=False, debug=False,
                   enable_asserts=False)
    xT = nc.dram_tensor("xT", [D, S], MM, kind="ExternalInput").ap()
    wq_t = nc.dram_tensor("wq_t", [D, C], MM, kind="ExternalInput").ap()
    wk_t = nc.dram_tensor("wk_t", [D, C], MM, kind="ExternalInput").ap()
    wv_t = nc.dram_tensor("wv_t", [D, C], MM, kind="ExternalInput").ap()
    wo2 = nc.dram_tensor("wo2", [P, 2, D], MM, kind="ExternalInput").ap()
    cos2 = nc.dram_tensor("cos2", [P, S], MM, kind="ExternalInput").ap()
    sin2 = nc.dram_tensor("sin2", [P, S], MM, kind="ExternalInput").ap()
    mask2 = nc.dram_tensor("mask2", [P, 2 * P], MM, kind="ExternalInput").ap()
    onesv = nc.dram_tensor("onesv", [P, NKT * HPC], MM,
                           kind="ExternalInput").ap()
    y = nc.dram_tensor("y", [S, D], F32, kind="ExternalOutput").ap()

    with tile.TileContext(nc) as tc:
        with tc.tile_pool(name="keep", bufs=1) as keep, \
             tc.tile_pool(name="wts", bufs=1) as wts, \
             tc.tile_pool(name="ph1", bufs=2) as ph1, \
             tc.tile_pool(name="swp", bufs=2) as swp, \
             tc.tile_pool(name="ptp", bufs=3) as ptp, \
             tc.tile_pool(name="normp", bufs=2) as normp, \
             tc.tile_pool(name="work", bufs=3) as work, \
             tc.tile_pool(name="psS", bufs=2, space="PSUM") as psS, \
             tc.tile_pool(name="psO", bufs=4, space="PSUM") as psO:

            # ------------ persistent tiles ------------
            # qk combo index: 0,1 = q pair0/1; 2,3 = k pair0/1
            qk = keep.tile([P, 4, S], MM, tag="qk")
            v_ext = keep.tile([P, NKT, HPC * (DH + 1)], MM, tag="vext")
            v4 = v_ext.rearrange("p t (h x) -> p t h x", h=HPC)
            mask_sb = keep.tile([P, 2, P], MM, tag="mask")
            attnT = [keep.tile([P, S], MM, tag=f"attnT{pr}", name=f"attnT{pr}")
                     for pr in range(2)]
            wo_sb = keep.tile([P, 2, D], MM, tag="wo")
            cos_sb = wts.tile([P, S], MM, tag="cos")
            sin_sb = wts.tile([P, S], MM, tag="sin")
            wq_sb = wts.tile([P, NE, C], MM, tag="wq")
            wk_sb = wts.tile([P, NE, C], MM, tag="wk")
            wv_sb = wts.tile([P, NE, C], MM, tag="wv")
            w_of = {"q": wq_sb, "k": wk_sb}

            # ------------ upfront loads (SP queue, priority order) ------------
            # wq/x0 chunked+interleaved so the first PE chain starts early;
            # everything else single-DMA to spare the HWDGE generator.
            xts = {0: ph1.tile([P, NE, QT], MM, tag="xt", name="xt0")}
            for e in range(NE):
                nc.sync.dma_start(
                    wq_sb[:, e], wq_t.rearrange("(o p) c -> p o c", p=P)[:, e])
                nc.sync.dma_start(
                    xts[0][:, e],
                    xT[:, 0:QT].rearrange("(o p) s -> p o s", p=P)[:, e])
            nc.sync.dma_start(wk_sb, wk_t.rearrange("(o p) c -> p o c", p=P))
            nc.sync.dma_start(wv_sb, wv_t.rearrange("(o p) c -> p o c", p=P))
            nc.sync.dma_start(cos_sb, cos2)
            nc.sync.dma_start(sin_sb, sin2)
            nc.sync.dma_start(mask_sb, mask2.rearrange("p (r c) -> p r c", r=2))
            nc.sync.dma_start(
                v4[:, :, :, DH:],
                onesv.rearrange("p (t h) -> p t h", t=NKT)[:, :, :, None])

            def phase1(st):
                sl = slice(st * QT, (st + 1) * QT)
                xt = xts.pop(st)
                for which in ("q", "k"):
                    slot = psS.tile([P, 2 * QT], F32, tag="ps", name="pqk")
                    for pr in range(2):
                        for e in range(NE):
                            nc.tensor.matmul(
                                slot[:, pr * QT:(pr + 1) * QT],
                                lhsT=w_of[which][:, e, pr * P:(pr + 1) * P],
                                rhs=xt[:, e],
                                start=(e == 0), stop=(e == NE - 1))
                    cb = 0 if which == "q" else 2
                    nc.scalar.copy(qk[:, cb:cb + 2, sl],
                                   slot.rearrange("p (r s) -> p r s", r=2))
                # RoPE half-rotation for q and k at once (ACT-queue DMAs so
                # they reach the DMA engines ahead of bulk prefetches)
                raw = qk[:, :, sl]
                sw = swp.tile([P, 4, QT], MM, tag="swap")
                for a in range(4):
                    src = (a ^ 1) * 32
                    nc.scalar.dma_start(sw[a * 32:(a + 1) * 32],
                                        raw[src:src + 32])
                if st + 1 < NQT:
                    nx = ph1.tile([P, NE, QT], MM, tag="xt")
                    xts[st + 1] = nx
                    nsl = slice((st + 1) * QT, (st + 2) * QT)
                    nc.scalar.dma_start(
                        nx, xT[:, nsl].rearrange("(o p) s -> p o s", p=P))
                if st == 1:
                    nc.scalar.dma_start(wo_sb, wo2)
                for cb in range(4):
                    nc.vector.tensor_tensor(sw[:, cb], sw[:, cb],
                                            sin_sb[:, sl], MUL)
                    nc.vector.tensor_tensor(raw[:, cb], raw[:, cb],
                                            cos_sb[:, sl], MUL)
                    nc.vector.tensor_tensor(raw[:, cb], raw[:, cb],
                                            sw[:, cb], ADD)
                vslot = psS.tile([P, 2 * QT], F32, tag="ps", name="pv")
                for sb16 in range(4):
                    for e in range(NE):
                        nc.tensor.matmul(
                            vslot[:, sb16 * C:(sb16 + 1) * C],
                            lhsT=xt[:, e, sb16 * P:(sb16 + 1) * P],
                            rhs=wv_sb[:, e],
                            start=(e == 0), stop=(e == NE - 1))
                nc.scalar.copy(
                    v4[:, st * 4:(st + 1) * 4, :, :DH],
                    vslot.rearrange("p (t h x) -> p t h x", t=4, h=HPC))

            def attention(qt):
                nkt = 4 * qt + 4
                for pr in range(2):
                    po = [psO.tile([P, QT], F32, tag="po",
                                   name=f"po{pr}{hh}") for hh in range(2)]
                    for kt in range(nkt):
                        j = kt - 4 * qt   # >= 0 on diagonal blocks
                        lo = max(j, 0) * P
                        slot = psS.tile([P, 2 * QT], F32, tag="ps",
                                        name="psc")
                        sv = slot.rearrange("p (r s) -> p r s", r=2)
                        for hh in range(2):
                            nc.tensor.matmul(
                                sv[:, hh, lo:],
                                lhsT=qk[hh * DH:(hh + 1) * DH, 2 + pr,
                                        kt * P:(kt + 1) * P],
                                rhs=qk[hh * DH:(hh + 1) * DH, pr,
                                       qt * QT + lo:(qt + 1) * QT])
                        pt = ptp.tile([P, 2, QT], MM, tag="pt")
                        nc.scalar.activation(pt[:, :, lo:], sv[:, :, lo:],
                                             EXP, scale=0.125)
                        if j >= 0:
                            nc.gpsimd.tensor_tensor(pt[:, :, lo:lo + P],
                                                    pt[:, :, lo:lo + P],
                                                    mask_sb, MUL)
                        for hh in range(2):
                            nc.tensor.matmul(
                                po[hh][:DH + 1, lo:],
                                lhsT=v4[:, kt, 2 * pr + hh],
                                rhs=pt[:, hh, lo:],
                                start=(kt == 0), stop=(kt == nkt - 1))
                    for hh in range(2):
                        den_r = normp.tile([1, QT], F32, tag="den")
                        with nc.allow_low_precision(reason="softmax"):
                            nc.vector.reciprocal(den_r, po[hh][DH:DH + 1])
                        den_b = normp.tile([DH, QT], F32, tag="dnb")
                        nc.gpsimd.partition_broadcast(den_b, den_r)
                        nc.vector.tensor_tensor(
                            attnT[pr][hh * DH:(hh + 1) * DH,
                                      qt * QT:(qt + 1) * QT],
                            po[hh][:DH], den_b, MUL)

            def phase3(qt):
                for sc in range(4 * qt, 4 * qt + 4):
                    y_sb = work.tile([P, D], F32, tag="ysb")
                    for et in range(D // QT):
                        psy = psO.tile([P, QT], F32, tag="po", name="psy")
                        for pr in range(2):
                            nc.tensor.matmul(
                                psy,
                                lhsT=attnT[pr][:, sc * P:(sc + 1) * P],
                                rhs=wo_sb[:, pr, et * QT:(et + 1) * QT],
                                start=(pr == 0), stop=(pr == 1))
                        if et == 0:
                            nc.vector.tensor_copy(y_sb[:, :QT], psy)
                        else:
                            nc.scalar.copy(y_sb[:, QT:], psy)
                    nc.sync.dma_start(y[sc * P:(sc + 1) * P], y_sb)

            phase1(0)
            for st in range(1, NQT):
                phase1(st)
                attention(st - 1)
                phase3(st - 1)
            attention(NQT - 1)
            phase3(NQT - 1)
    nc.compile()
    return nc


def _get_nc():
    if "nc" not in _cache:
        _cache["nc"] = _build()
    return _cache["nc"]


def _host_inputs(x, Wq, Wk, Wv, Wo, cos, sin):
    """Build the 8 per-core input dicts."""
    cosT = np.ascontiguousarray(cos.T).astype(np.float32)     # [DH, S]
    sinT = np.ascontiguousarray(sin.T).astype(np.float32)
    sinS = np.concatenate([-sinT[:DH // 2], sinT[DH // 2:]], axis=0)
    cos2 = np.tile(cosT, (2, 1))                              # [128, S]
    sin2 = np.tile(sinS, (2, 1))
    mask1 = (np.arange(P)[:, None] <= np.arange(P)[None, :]).astype(np.float32)
    mask2 = np.tile(mask1, (1, 2))                            # [128, 256]
    onesv = np.ones((P, NKT * HPC), np.float32)

    in_maps = []
    for c in range(NCORES):
        b, g = divmod(c, 4)
        cs = slice(C * g, C * g + C)
        in_maps.append({
            "xT": np.ascontiguousarray(x[b].T).astype(np.float32),
            "wq_t": np.ascontiguousarray(Wq[cs].T).astype(np.float32),
            "wk_t": np.ascontiguousarray(Wk[cs].T).astype(np.float32),
            "wv_t": np.ascontiguousarray(Wv[cs].T).astype(np.float32),
            "wo2": np.ascontiguousarray(
                Wo.T[cs].reshape(2, P, D).transpose(1, 0, 2)
            ).astype(np.float32),
            "cos2": cos2, "sin2": sin2, "mask2": mask2, "onesv": onesv,
        })
    return in_maps


def run(x, Wq, Wk, Wv, Wo, cos, sin, mask=None, trace=False, **trace_kw):
    import os
    import time
    if not trace:
        # The axon NTFF-profile hook is not installed in all containers;
        # make sure an inherited BASS_TRACE=1 can't send us down that path.
        os.environ.setdefault("BASS_NEVER_TRACE", "1")
    from concourse.bass_utils import run_bass_kernel_spmd
    nc = _get_nc()
    in_maps = _host_inputs(np.asarray(x), np.asarray(Wq), np.asarray(Wk),
                           np.asarray(Wv), np.asarray(Wo), np.asarray(cos),
                           np.asarray(sin))
    try:
        res = run_bass_kernel_spmd(nc, in_maps, core_ids=list(range(NCORES)),
                                   trace=trace, **trace_kw)
    except Exception:
        # one retry for transient device states (e.g. NRT_EXEC_UNIT errors)
        time.sleep(15)
        res = run_bass_kernel_spmd(nc, in_maps, core_ids=list(range(NCORES)),
                                   trace=trace, **trace_kw)
    parts = [r["y"] for r in res.results]
    out = np.stack([parts[0] + parts[1] + parts[2] + parts[3],
                    parts[4] + parts[5] + parts[6] + parts[7]])
    return out.astype(np.float32), res


def kernel(x, Wq, Wk, Wv, Wo, cos, sin, mask=None, **_):
    out, _res = run(x, Wq, Wk, Wv, Wo, cos, sin, mask)
    return out
